# revision 1
# baseline (speedup 1.0000x reference)
"""Trainium2 Bass kernel for an AttentionBlock (GroupNorm -> 1x1 qkv ->
full HxW self-attention -> 1x1 proj -> residual).

Contract: kernel(**inputs) takes FULL unsharded numpy inputs (see shapes
below) and returns the FULL output [32, 512, 32, 32] float32.

Sharding: data-parallel over batch B=32 across 8 NeuronCores (4 samples
per core). No collectives.

Per-core algorithm (C=512 channels, HW=1024 pixels per sample):
  - GroupNorm(32 groups): per-channel sum / sum-of-squares on DVE/ACT,
    cross-partition group reduction via tiny PE indicator matmuls,
    normalize as a per-channel affine on ACT.
  - scores: s = (Wk~ h).T (Wq~ h) = h.T M h with M = Wk~.T Wq~ built on
    the HOST (weights only, zero qkv bias fast path) — on-chip we do
    A = M.T h then sT = A.T h, eliminating separate q/k tensors.
    v is produced directly transposed (vT = [pix, chan]) by swapping
    matmul operands, so no on-chip transposes anywhere.
  - softmax matches the reference's axis=-1 quirk: each 32-key row is
    normalized independently. exp() on ACT without max-subtraction
    (logits are O(1) by construction); blockwise normalizers via
    indicator matmuls into one [32, 512] PSUM per chunk, one batched
    reciprocal, indicator broadcast-back, e scaled in place.
  - h~[c,i] = sum_j vT[j,c] e_norm[j,i]; proj; out = proj + proj_b + x.

Big matmuls run bf16 (fp32 PSUM accumulation, fp32 residual); GroupNorm
stat matmuls run float32r. Samples are software-pipelined: sample s+1's
x-loads/stats, group-matmuls/apply, and A/v production are emitted at
staggered points inside sample s's attention so the PE stream stays
dense.
"""

import sys
from contextlib import ExitStack

for _p in ("/opt/trn_rl_repo", "/root/.axon_site/_ro/trn_rl_repo"):
    if _p not in sys.path:
        sys.path.insert(0, _p)

import numpy as np
import ml_dtypes

BF16_NP = ml_dtypes.bfloat16

import concourse.bass as bass  # noqa: F401  (registers AP machinery)
import concourse.mybir as mybir
import concourse.tile as tile
from concourse import bacc
from concourse.bass_utils import run_bass_kernel_spmd

F32 = mybir.dt.float32
F32R = mybir.dt.float32r
BF16 = mybir.dt.bfloat16
MMDT = BF16  # dtype of the big matmul operands
AF = mybir.ActivationFunctionType
ALU = mybir.AluOpType
AX = mybir.AxisListType

N_CORES = 8
B = 32
C = 512
HW = 1024  # 32*32 pixels
BS = B // N_CORES  # samples per core
GROUPS = 32
GSIZE = C // GROUPS  # 16 channels per group
EPS = 1e-5
P = 128
CT = C // P  # 4 channel tiles
JT = HW // P  # 8 pixel tiles
NCH = 512  # i-chunk width (free dim per matmul)
NCHUNKS = HW // NCH  # 2
GPT = P // GSIZE  # groups per channel-tile = 8
GROUP_N = GSIZE * HW  # elements per group = 16384

_CACHE = {}


def _build(with_vbias=True, fuse_qk=True):
    """Build + compile the per-core Bass program. Returns nc."""
    nc = bacc.Bacc("TRN2", target_bir_lowering=False, debug=True)

    x_d = nc.dram_tensor("x", [BS, C, HW], F32, kind="ExternalInput")
    xb_d = nc.dram_tensor("xbf", [BS, C, HW], MMDT, kind="ExternalInput")
    wq_d = nc.dram_tensor("wqT", [C, C], MMDT, kind="ExternalInput")
    mm_d = nc.dram_tensor("mqk", [C, C], MMDT, kind="ExternalInput")
    wk_d = nc.dram_tensor("wkT", [C, C], MMDT, kind="ExternalInput")
    wv_d = nc.dram_tensor("wvT", [C, C], MMDT, kind="ExternalInput")
    wp_d = nc.dram_tensor("wpT", [C, C], MMDT, kind="ExternalInput")
    bq_d = nc.dram_tensor("bq", [C], F32, kind="ExternalInput")
    bk_d = nc.dram_tensor("bk", [C], F32, kind="ExternalInput")
    bv_d = nc.dram_tensor("bv", [C], MMDT, kind="ExternalInput")
    pb_d = nc.dram_tensor("pb", [C], F32, kind="ExternalInput")
    gam_d = nc.dram_tensor("gamma", [C], F32, kind="ExternalInput")
    bet_d = nc.dram_tensor("beta", [C], F32, kind="ExternalInput")
    gm_d = nc.dram_tensor("gmat", [P, GPT], F32R, kind="ExternalInput")
    onr_d = nc.dram_tensor("onesr", [1, P], MMDT, kind="ExternalInput")
    b1_d = nc.dram_tensor("b1all", [P, JT, 32], MMDT, kind="ExternalInput")
    b2_d = nc.dram_tensor("b2all", [32, JT, P], MMDT, kind="ExternalInput")
    gmt_d = nc.dram_tensor("gmatT", [GPT, P], F32R, kind="ExternalInput")
    out_d = nc.dram_tensor("out", [BS, C, HW], F32, kind="ExternalOutput")

    with tile.TileContext(nc) as tc, ExitStack() as ctx:
        ctx.enter_context(nc.allow_low_precision(
            reason="bf16/float32r matmul operands are rounded; all "
                   "accumulations are fp32 (PSUM / fp32 stat tiles)"))
        ep_ = ctx.enter_context
        const = ep_(tc.tile_pool(name="const", bufs=1))
        xp = ep_(tc.tile_pool(name="xp", bufs=2))
        xbp = ep_(tc.tile_pool(name="xbp", bufs=2))
        hp = ep_(tc.tile_pool(name="hp", bufs=2))
        qp = ep_(tc.tile_pool(name="qp", bufs=2))
        kp = ep_(tc.tile_pool(name="kp", bufs=2))
        vp = ep_(tc.tile_pool(name="vp", bufs=2))
        ep = ep_(tc.tile_pool(name="ep", bufs=3))
        htp = ep_(tc.tile_pool(name="htp", bufs=2))
        outp = ep_(tc.tile_pool(name="outp", bufs=4))
        sqp = ep_(tc.tile_pool(name="sqp", bufs=2))
        statp = ep_(tc.tile_pool(name="statp", bufs=4))
        gnp = ep_(tc.tile_pool(name="gnp", bufs=2))
        rrp = ep_(tc.tile_pool(name="rrp", bufs=2))
        ps_mm = ep_(tc.tile_pool(name="ps_mm", bufs=6, space="PSUM"))
        ps_zr = ep_(tc.tile_pool(name="ps_zr", bufs=2, space="PSUM"))
        if True:
            # ---- constants ----
            wq_sb = const.tile([P, CT, C], MMDT, tag="wq")
            wk_sb = const.tile([P, CT, C], MMDT, tag="wk")
            wv_sb = const.tile([P, CT, C], MMDT, tag="wv")
            wp_sb = const.tile([P, CT, C], MMDT, tag="wp")
            bq_sb = const.tile([P, CT], F32, tag="bq")
            bk_sb = const.tile([P, CT], F32, tag="bk")
            pb_sb = const.tile([P, CT], F32, tag="pb")
            gam_sb = const.tile([P, CT], F32, tag="gam")
            bet_sb = const.tile([P, CT], F32, tag="bet")
            for t_sb, t_d in ((bq_sb, bq_d), (bk_sb, bk_d), (pb_sb, pb_d),
                              (gam_sb, gam_d), (bet_sb, bet_d)):
                nc.gpsimd.dma_start(out=t_sb, in_=t_d.rearrange("(t p) -> p t", p=P))
            bv_sb = const.tile([1, C], MMDT, tag="bv")
            nc.gpsimd.dma_start(out=bv_sb, in_=bv_d[None, :])
            gm_sb = const.tile([P, GPT], F32R, tag="gm")
            nc.gpsimd.dma_start(out=gm_sb, in_=gm_d[:, :])
            gmt_sb = const.tile([GPT, P], F32R, tag="gmt")
            nc.gpsimd.dma_start(out=gmt_sb, in_=gmt_d[:, :])
            ones_row = const.tile([1, P], MMDT, tag="oner")
            nc.gpsimd.dma_start(out=ones_row, in_=onr_d[:, :])
            b1_sb = const.tile([P, JT, 32], MMDT, tag="b1")
            nc.gpsimd.dma_start(out=b1_sb, in_=b1_d[:, :, :])
            b2_sb = const.tile([32, JT, P], MMDT, tag="b2")
            nc.gpsimd.dma_start(out=b2_sb, in_=b2_d[:, :, :])
            eps_sb = const.tile([P, 1], F32, tag="eps")
            nc.vector.memset(eps_sb, EPS)
            mm_sb = const.tile([P, CT, C], MMDT, tag="mqk")
            # big weight DMAs last: the small consts above gate the first
            # sample's GN matmuls and must not queue behind 2MB of weights
            big = ((mm_sb, mm_d), (wv_sb, wv_d), (wp_sb, wp_d)) if fuse_qk \
                else ((wq_sb, wq_d), (wk_sb, wk_d), (wv_sb, wv_d), (wp_sb, wp_d))
            for t_sb, t_d in big:
                nc.gpsimd.dma_start(out=t_sb, in_=t_d.rearrange("(t p) o -> p t o", p=P))

            def emit_gn_load(s):
                """x loads + per-channel stats (DMA/DVE/ACT only) —
                issued a phase early so the packet-rate-limited fp32 x
                transfer completes before the normalize apply needs it."""
                xb_t = []
                for t in range(CT):
                    xbt = xbp.tile([P, HW], MMDT, tag=f"xb{t}")
                    nc.sync.dma_start(
                        out=xbt, in_=xb_d[s, t * P:(t + 1) * P, :])
                    xb_t.append(xbt)
                xs_t = []
                for t in range(CT):
                    xt = xp.tile([P, HW], F32, tag=f"xs{t}")
                    nc.sync.dma_start(
                        out=xt, in_=x_d[s, t * P:(t + 1) * P, :])
                    xs_t.append(xt)
                stats = []
                for t in range(CT):
                    stat = statp.tile([P, 2], F32, tag="stat")
                    nc.vector.tensor_reduce(
                        out=stat[:, 0:1], in_=xb_t[t], axis=AX.X, op=ALU.add)
                    sq = sqp.tile([P, HW], MMDT, tag="sq")
                    nc.scalar.activation(
                        out=sq, in_=xb_t[t], func=AF.Square,
                        accum_out=stat[:, 1:2])
                    stat_r = statp.tile([P, 2], F32R, tag="stat_r")
                    nc.vector.tensor_copy(out=stat_r, in_=stat)
                    stats.append(stat_r)
                return xb_t, xs_t, stats

            def emit_gn_finish(gn_ld):
                """Group matmuls + affine + normalize apply."""
                xb_t, xs_t, stats = gn_ld
                pgs = ps_zr.tile([GPT, 2 * CT], F32, tag="pz32")
                for t in range(CT):
                    nc.tensor.matmul(
                        pgs[:, 2 * t:2 * t + 2], lhsT=gm_sb[:, :],
                        rhs=stats[t][:, :], start=True, stop=True)
                packed = gnp.tile([GPT, 2 * CT], F32R, tag="packed")
                nc.vector.tensor_scalar_mul(
                    packed[:, 0:CT], pgs[:, 0:2 * CT:2], 1.0 / GROUP_N)
                ex2 = gnp.tile([GPT, CT], F32, tag="ex2")
                nc.vector.tensor_scalar_mul(
                    ex2, pgs[:, 1:2 * CT:2], 1.0 / GROUP_N)
                msq = gnp.tile([GPT, CT], F32, tag="msq")
                nc.vector.tensor_tensor(
                    out=msq, in0=packed[:, 0:CT], in1=packed[:, 0:CT],
                    op=ALU.mult)
                nc.vector.tensor_tensor(
                    out=packed[:, CT:2 * CT], in0=ex2, in1=msq,
                    op=ALU.subtract)
                nc.scalar.activation(
                    out=packed[:, CT:2 * CT], in_=packed[:, CT:2 * CT],
                    func=AF.Sqrt, bias=eps_sb[0:GPT, :], scale=1.0)
                nc.vector.reciprocal(
                    out=packed[:, CT:2 * CT], in_=packed[:, CT:2 * CT])
                mv = gnp.tile([P, CT, 2], F32, tag="mv")
                for t in range(CT):
                    pbc = ps_zr.tile([P, 2], F32, tag="pz32")
                    nc.tensor.matmul(
                        pbc, lhsT=gmt_sb[:, :],
                        rhs=packed[:, t::CT], start=True, stop=True)
                    nc.vector.tensor_copy(out=mv[:, t, :], in_=pbc)
                sc_all = gnp.tile([P, CT], F32, tag="sc_all")
                nc.vector.tensor_tensor(
                    out=sc_all, in0=mv[:, :, 1], in1=gam_sb, op=ALU.mult)
                tmp_all = gnp.tile([P, CT], F32, tag="tmp_all")
                nc.vector.tensor_tensor(
                    out=tmp_all, in0=mv[:, :, 0], in1=sc_all, op=ALU.mult)
                toff_all = gnp.tile([P, CT], F32, tag="toff_all")
                nc.vector.tensor_tensor(
                    out=toff_all, in0=bet_sb, in1=tmp_all, op=ALU.subtract)
                hs = hp.tile([P, CT, HW], MMDT, tag="hs")
                for t in range(CT):
                    nc.scalar.activation(
                        out=hs[:, t, :], in_=xs_t[t], func=AF.Identity,
                        bias=toff_all[:, t:t + 1], scale=sc_all[:, t:t + 1])
                return xs_t, hs

            def emit_qkv(hs):
                """Score operands + vT.

                fuse_qk: A = (Wk~.T Wq~).T h — scores are then A.T @ h,
                eliminating separate q and k (saves 32 matmuls/sample).
                Otherwise classic q/k with per-channel biases."""
                if fuse_qk:
                    qs = hs  # scores' rhs is h itself
                    ks = kp.tile([P, CT, HW], MMDT, tag="ks")  # A
                    for m in range(CT):
                        for h in range(NCHUNKS):
                            pq = ps_mm.tile([P, NCH], F32, tag="pmm")
                            for kk in range(CT):
                                nc.tensor.matmul(
                                    pq,
                                    lhsT=mm_sb[:, kk, m * P:(m + 1) * P],
                                    rhs=hs[:, kk, h * NCH:(h + 1) * NCH],
                                    start=(kk == 0), stop=(kk == CT - 1))
                            if (m + h) % 2 == 0:
                                nc.scalar.copy(
                                    ks[:, m, h * NCH:(h + 1) * NCH], pq)
                            else:
                                nc.vector.tensor_copy(
                                    out=ks[:, m, h * NCH:(h + 1) * NCH],
                                    in_=pq)
                else:
                    qs = qp.tile([P, CT, HW], MMDT, tag="qs")
                    ks = kp.tile([P, CT, HW], MMDT, tag="ks")
                    for di, (dst, w_sb, b_sb) in enumerate(
                            ((qs, wq_sb, bq_sb), (ks, wk_sb, bk_sb))):
                        for m in range(CT):
                            for h in range(NCHUNKS):
                                pq = ps_mm.tile([P, NCH], F32, tag="pmm")
                                for kk in range(CT):
                                    nc.tensor.matmul(
                                        pq,
                                        lhsT=w_sb[:, kk, m * P:(m + 1) * P],
                                        rhs=hs[:, kk, h * NCH:(h + 1) * NCH],
                                        start=(kk == 0), stop=(kk == CT - 1))
                                if (m + h + di) % 2 == 0:
                                    nc.scalar.activation(
                                        out=dst[:, m, h * NCH:(h + 1) * NCH],
                                        in_=pq, func=AF.Identity,
                                        bias=b_sb[:, m:m + 1])
                                else:
                                    nc.vector.tensor_scalar_add(
                                        dst[:, m, h * NCH:(h + 1) * NCH], pq,
                                        b_sb[:, m:m + 1])
                vts = vp.tile([P, JT, C], MMDT, tag="vts")
                for m in range(JT):
                    pv = ps_mm.tile([P, NCH], F32, tag="pmm")
                    for kk in range(CT):
                        nc.tensor.matmul(
                            pv, lhsT=hs[:, kk, m * P:(m + 1) * P],
                            rhs=wv_sb[:, kk, :],
                            start=(kk == 0),
                            stop=(not with_vbias and kk == CT - 1))
                    if with_vbias:
                        nc.tensor.matmul(
                            pv, lhsT=ones_row, rhs=bv_sb,
                            start=False, stop=True)
                    nc.scalar.copy(vts[:, m, :], pv)
                return qs, ks, vts

            xs_cur, hs_cur = emit_gn_finish(emit_gn_load(0))
            qkv_cur = emit_qkv(hs_cur)
            hs_nxt = None
            for s in range(BS):
                xs_t = xs_cur
                qs, ks, vts = qkv_cur
                gn_ld = None

                # ---- attention phase A: scores+exp+blockwise Z, both chunks
                es_c = []
                pz_c = []
                for h in range(NCHUNKS):
                    isl = slice(h * NCH, (h + 1) * NCH)
                    es = ep.tile([P, JT, NCH], MMDT, tag="es")
                    pz32 = ps_zr.tile([32, NCH], F32, tag="pz32")
                    for j in range(JT):
                        psj = ps_mm.tile([P, NCH], F32, tag="pmm")
                        for kk in range(CT):
                            nc.tensor.matmul(
                                psj, lhsT=ks[:, kk, j * P:(j + 1) * P],
                                rhs=qs[:, kk, isl],
                                start=(kk == 0), stop=(kk == CT - 1))
                        nc.scalar.activation(
                            out=es[:, j, :], in_=psj, func=AF.Exp)
                        if j > 0:
                            nc.tensor.matmul(
                                pz32, lhsT=b1_sb[:, j - 1, :],
                                rhs=es[:, j - 1, :],
                                start=(j - 1 == 0), stop=False,
                                skip_group_check=True)
                    nc.tensor.matmul(
                        pz32, lhsT=b1_sb[:, JT - 1, :],
                        rhs=es[:, JT - 1, :],
                        start=False, stop=True,
                        skip_group_check=True)
                    es_c.append(es)
                    pz_c.append(pz32)
                    if h == 0 and s + 1 < BS:
                        gn_ld = emit_gn_load(s + 1)

                # next sample's GN group-matmuls/apply at the attention
                # midpoint; its loads/stats were issued a phase earlier.
                if gn_ld is not None:
                    xs_cur, hs_nxt = emit_gn_finish(gn_ld)

                # ---- attention phase B; qkv of s+1 emitted between the
                # two chunks so PE has dense independent work to absorb
                # normalize-chain handoffs ----
                for h in range(NCHUNKS):
                    if h == 1 and s + 1 < BS:
                        qkv_cur = emit_qkv(hs_nxt)
                    isl = slice(h * NCH, (h + 1) * NCH)
                    es, pz32 = es_c[h], pz_c[h]
                    rr32 = rrp.tile([32, NCH], MMDT, tag="rr32")
                    nc.vector.reciprocal(out=rr32, in_=pz32)
                    for j in range(JT):
                        prb = ps_mm.tile([P, NCH], F32, tag="pmm")
                        nc.tensor.matmul(
                            prb, lhsT=b2_sb[:, j, :], rhs=rr32,
                            start=True, stop=True)
                        nc.vector.tensor_tensor(
                            out=es[:, j, :], in0=es[:, j, :], in1=prb,
                            op=ALU.mult)
                    hts = htp.tile([P, CT, NCH], MMDT, tag="hts")
                    for m in range(CT):
                        ph = ps_mm.tile([P, NCH], F32, tag="pmm")
                        for j in range(JT):
                            nc.tensor.matmul(
                                ph, lhsT=vts[:, j, m * P:(m + 1) * P],
                                rhs=es[:, j, :],
                                start=(j == 0), stop=(j == JT - 1))
                        nc.scalar.copy(hts[:, m, :], ph)
                    for m in range(CT):
                        pp = ps_mm.tile([P, NCH], F32, tag="pmm")
                        for kk in range(CT):
                            nc.tensor.matmul(
                                pp, lhsT=wp_sb[:, kk, m * P:(m + 1) * P],
                                rhs=hts[:, kk, :],
                                start=(kk == 0), stop=(kk == CT - 1))
                        ot = outp.tile([P, NCH], F32, tag="ot")
                        nc.vector.scalar_tensor_tensor(
                            out=ot, in0=pp, scalar=pb_sb[:, m:m + 1],
                            in1=xs_t[m][:, isl], op0=ALU.add, op1=ALU.add)
                        nc.sync.dma_start(
                            out=out_d[s, m * P:(m + 1) * P, isl], in_=ot)

    nc.compile()
    return nc


def _get_nc(with_vbias=True, fuse_qk=True):
    key = ("nc", with_vbias, fuse_qk)
    if key not in _CACHE:
        _CACHE[key] = _build(with_vbias, fuse_qk)
    return _CACHE[key]


def kernel(x, gn_gamma, gn_beta, qkv_w, qkv_b, proj_w, proj_b, _trace=False):
    x = np.ascontiguousarray(np.asarray(x, dtype=np.float32))
    qkv_w = np.asarray(qkv_w, dtype=np.float32)
    qkv_b = np.asarray(qkv_b, dtype=np.float32)
    proj_w = np.asarray(proj_w, dtype=np.float32)
    proj_b = np.asarray(proj_b, dtype=np.float32)
    gn_gamma = np.asarray(gn_gamma, dtype=np.float32)
    gn_beta = np.asarray(gn_beta, dtype=np.float32)

    scale = 1.0 / np.sqrt(np.sqrt(np.float32(C)))  # applied to q AND k
    wqT = np.ascontiguousarray((qkv_w[0:C] * scale).T).astype(BF16_NP)
    wkT = np.ascontiguousarray((qkv_w[C:2 * C] * scale).T).astype(BF16_NP)
    wvT = np.ascontiguousarray(qkv_w[2 * C:3 * C].T).astype(BF16_NP)
    wpT = np.ascontiguousarray(proj_w.T).astype(BF16_NP)
    bq = np.ascontiguousarray(qkv_b[0:C] * scale)
    bk = np.ascontiguousarray(qkv_b[C:2 * C] * scale)
    bv = np.ascontiguousarray(qkv_b[2 * C:3 * C]).astype(BF16_NP)
    pb = np.ascontiguousarray(proj_b)

    wq_s = qkv_w[0:C] * scale
    wk_s = qkv_w[C:2 * C] * scale
    mqk = np.ascontiguousarray((wk_s.T @ wq_s)).astype(BF16_NP)

    cidx = np.arange(P)
    gmat = (cidx[:, None] // GSIZE == np.arange(GPT)[None, :]).astype(np.float32)
    b1all = np.zeros((P, JT, 32), np.float32)
    b2all = np.zeros((32, JT, P), np.float32)
    for jt in range(JT):
        for p_ in range(P):
            r = 4 * jt + p_ // 32
            b1all[p_, jt, r] = 1.0
            b2all[r, jt, p_] = 1.0
    gmatT = np.ascontiguousarray(gmat.T)

    xs = x.reshape(B, C, HW)
    common = dict(wqT=wqT, wkT=wkT, wvT=wvT, wpT=wpT, mqk=mqk,
                  bq=bq, bk=bk, bv=bv,
                  pb=pb, gamma=gn_gamma, beta=gn_beta, gmat=gmat, gmatT=gmatT,
                  onesr=np.ones((1, P), BF16_NP),
                  b1all=b1all.astype(BF16_NP), b2all=b2all.astype(BF16_NP))
    xbf = xs.astype(BF16_NP)
    in_maps = [
        {"x": np.ascontiguousarray(xs[i * BS:(i + 1) * BS]),
         "xbf": np.ascontiguousarray(xbf[i * BS:(i + 1) * BS]), **common}
        for i in range(N_CORES)
    ]

    fuse_qk = not (np.any(bq) or np.any(bk))
    nc = _get_nc(with_vbias=bool(np.any(np.asarray(bv, np.float32))),
                 fuse_qk=fuse_qk)
    try:
        res = run_bass_kernel_spmd(
            nc, in_maps, core_ids=list(range(N_CORES)), trace=_trace)
    except Exception:
        res = run_bass_kernel_spmd(
            nc, in_maps, core_ids=list(range(N_CORES)), trace=_trace)
    _CACHE["last_result"] = res
    out = np.concatenate([res.results[i]["out"] for i in range(N_CORES)], axis=0)
    return out.reshape(B, C, 32, 32).astype(np.float32, copy=False)

